# revision 1
# baseline (speedup 1.0000x reference)
"""Causal attention (B=4, S=2048, D=1024) on 8 Trainium2 NeuronCores.

Sharding: data-parallel over batch (4) x query-block-parallel (2 cores per
batch).  Global q-tiles (128 rows each, 16 per batch) are dealt round-robin:
core h=0 of a pair takes even tiles, h=1 odd tiles.  The program rounds every
q-tile's causal key-extent up to a multiple of 256 -- tile pair (2j, 2j+1)
then shares the extent 256*(j+1), so both cores run the *same* instruction
stream (SPMD) and the residual causal masking is supplied as a per-core
additive-mask input.  K/V are computed (duplicated) per pair from the full
batch sequence; no cross-device communication (a pair AllGather was measured
at ~60us fixed latency -- more than the duplicated projection work it saves).

DMA pipelining: wq has its own pool and wk/wv their own two slots so weight
casts never wait on each other's last matmul; the first key-chunk of x is
prefetched during the Q phase; 4-deep fp32 staging keeps the DMA queues fed.

All matmuls run in bf16 with fp32 PSUM accumulation:
  xT/xqT  : PE-transposed activations (d on partitions)
  QT[e,q] = wq^T xq^T / sqrt(D), KT[e,k] = wk^T x^T, V[k,e] = x wv
  S[q,k]  = QT^T KT (chunks of <=512 cols in PSUM), + additive mask tail
  P       = exp(S) (scores are O(1) -- max-subtraction is unnecessary),
            fused row-sum via activation accum_out
  O[q,e]  = (P^T)^T V accumulated over 128-key tiles, scaled by 1/rowsum
"""

import os

os.environ.setdefault("MYCRO_LOCAL_CACHE", "1")

import numpy as np

import concourse.bacc as bacc
import concourse.tile as tile
from concourse import mybir
from concourse.bass_utils import run_bass_kernel_spmd
from concourse.masks import make_identity

B, S, D = 4, 2048, 1024
P = 128
QL = S // 2          # queries per core
NCORES = 8
DT = D // P          # 8 d-tiles (contraction)
ET = D // P          # 8 e-tiles
ST = S // P          # 16 s-tiles
NQT = QL // P        # 8 q-tiles per core
F32 = mybir.dt.float32
BF16 = mybir.dt.bfloat16
NEG = -30000.0       # additive mask value; exp() underflows to exactly 0


def _chunks(extent):
    out, o = [], 0
    while o < extent:
        w = min(512, extent - o)
        out.append((o, w))
        o += w
    return out


def _body(tc, x, xq, wq, wk, wv, mask, out):
    nc = tc.nc
    with (
        tc.tile_pool(name="consts", bufs=1) as consts,
        tc.tile_pool(name="qkv", bufs=1) as qkv,
    ):
        ident = consts.tile([P, P], BF16)
        make_identity(nc, ident)
        mask_sb = consts.tile([P, 256], F32)
        nc.sync.dma_start(mask_sb, mask)

        qT = qkv.tile([P, ET, QL], BF16)   # [e_in, e_tile, q]
        kT = qkv.tile([P, ET, S], BF16)    # [e_in, e_tile, k]
        v = qkv.tile([P, ST, D], BF16)     # [k_in, k_tile, e]

        # ------------------------------ projections ------------------------
        outer = tc.tile_pool(name="pmm", bufs=4, space="PSUM")
        pmm = outer.__enter__()
        with (
            tc.tile_pool(name="wqp", bufs=1) as wqp,
            tc.tile_pool(name="wsb", bufs=2) as wpool,
            tc.tile_pool(name="stage", bufs=4) as stpool,
            tc.tile_pool(name="castq", bufs=3) as castq,
            tc.tile_pool(name="castx", bufs=6) as castx,
            tc.tile_pool(name="ptr", bufs=4, space="PSUM") as ptr,
        ):
            def load_weight(w_ap, pool):
                wsb = pool.tile([P, DT, D], BF16, tag="w")
                for d in range(DT):
                    stg = stpool.tile([P, D], F32, tag="stage")
                    nc.sync.dma_start(stg, w_ap[d * P:(d + 1) * P, :])
                    nc.vector.tensor_copy(wsb[:, d, :], stg)
                return wsb

            def load_cast(x_ap, s, cpool, split=False):
                stg = stpool.tile([P, D], F32, tag="stage")
                xb = cpool.tile([P, D], BF16, tag="cast")
                if split:
                    for h_ in range(2):
                        cols = slice(h_ * (D // 2), (h_ + 1) * (D // 2))
                        nc.sync.dma_start(stg[:, cols],
                                          x_ap[s * P:(s + 1) * P, cols])
                        nc.vector.tensor_copy(xb[:, cols], stg[:, cols])
                else:
                    nc.sync.dma_start(stg, x_ap[s * P:(s + 1) * P, :])
                    nc.vector.tensor_copy(xb, stg)
                return xb

            def transpose_into(xb, s, dst):
                # dst[:, d, s*128:(s+1)*128] = tile.T (bf16)
                for d in range(DT):
                    pst = ptr.tile([P, P], BF16, tag="tp")
                    nc.tensor.transpose(pst, xb[:, d * P:(d + 1) * P], ident)
                    nc.vector.tensor_copy(dst[:, d, s * P:(s + 1) * P], pst)

            # ---- Q phase (with x chunk-0 DMA prefetch folded in)
            x_pref = {}
            wq_sb = None
            with tc.tile_pool(name="xqp", bufs=1) as xqp:
                xqT = xqp.tile([P, DT, QL], BF16, tag="xqT")
                for c in range(QL // 512):
                    for s in range(4 * c, 4 * c + 4):
                        xb = load_cast(xq, s, castq, split=(s == 0))
                        transpose_into(xb, s, xqT)
                    if wq_sb is None:
                        wq_sb = load_weight(wq, wqp)
                    if c == 0:
                        for s in range(4):
                            x_pref[s] = load_cast(x, s, castx)
                    else:
                        for s in range(4, 6):
                            x_pref[s] = load_cast(x, s, castx)
                    for e in range(ET):
                        ps = pmm.tile([P, 512], F32, tag="mm")
                        for d in range(DT):
                            nc.tensor.matmul(
                                ps, wq_sb[:, d, e * P:(e + 1) * P],
                                xqT[:, d, c * 512:(c + 1) * 512],
                                start=(d == 0), stop=(d == DT - 1))
                        nc.scalar.mul(qT[:, e, c * 512:(c + 1) * 512], ps,
                                      1.0 / 32.0)

            # ---- K/V phase
            wk_sb = load_weight(wk, wpool)
            wv_sb = load_weight(wv, wpool)
            with tc.tile_pool(name="xtp", bufs=1) as xtp:
                xT = xtp.tile([P, DT, S], BF16, tag="xT")
                for c in range(S // 512):
                    for s in range(4 * c, 4 * c + 4):
                        xb = x_pref.pop(s, None)
                        if xb is None:
                            xb = load_cast(x, s, castx)
                        transpose_into(xb, s, xT)
                    # KT[e, k] = sum_d wk[d, e] x[k, d]
                    for e in range(ET):
                        ps = pmm.tile([P, 512], F32, tag="mm")
                        for d in range(DT):
                            nc.tensor.matmul(
                                ps, wk_sb[:, d, e * P:(e + 1) * P],
                                xT[:, d, c * 512:(c + 1) * 512],
                                start=(d == 0), stop=(d == DT - 1))
                        nc.scalar.copy(kT[:, e, c * 512:(c + 1) * 512], ps)
                    # V[k, e] = sum_d x[k, d] wv[d, e]
                    for k in range(4 * c, 4 * c + 4):
                        for ec in range(D // 512):
                            ps = pmm.tile([P, 512], F32, tag="mm")
                            for d in range(DT):
                                nc.tensor.matmul(
                                    ps, xT[:, d, k * P:(k + 1) * P],
                                    wv_sb[:, d, ec * 512:(ec + 1) * 512],
                                    start=(d == 0), stop=(d == DT - 1))
                            nc.scalar.copy(v[:, k, ec * 512:(ec + 1) * 512],
                                           ps)

        # ------------------------------ attention --------------------------
        with (
            tc.tile_pool(name="attn", bufs=3) as apool,
            tc.tile_pool(name="ptsb", bufs=6) as ptpool,
            tc.tile_pool(name="stats", bufs=2) as spool,
            tc.tile_pool(name="psT", bufs=2, space="PSUM") as psT,
            tc.tile_pool(name="psO", bufs=1, space="PSUM") as psO,
        ):
            psS = pmm
            for j in (7, 6, 5, 0, 4, 1, 3, 2):
                ext = 256 * (j + 1)
                chunks = _chunks(ext)
                p_sb = apool.tile([P, ext], BF16, tag="p")
                lsum = spool.tile([P, ext // P], F32, tag="lsum")
                for ci, (o, w) in enumerate(chunks):
                    ps = psS.tile([P, w], F32, tag="mm")
                    for e in range(ET):
                        nc.tensor.matmul(
                            ps, qT[:, e, j * P:(j + 1) * P], kT[:, e, o:o + w],
                            start=(e == 0), stop=(e == ET - 1))
                    if o + w == ext:
                        nc.vector.tensor_add(
                            ps[:, w - 256:w], ps[:, w - 256:w], mask_sb)
                    # 128-wide exp subtiles: each P^T transpose can start as
                    # soon as its own columns are exponentiated
                    for si in range(w // P):
                        col = o + si * P
                        nc.scalar.activation(
                            p_sb[:, col:col + P], ps[:, si * P:(si + 1) * P],
                            mybir.ActivationFunctionType.Exp,
                            accum_out=lsum[:, col // P:col // P + 1])
                l_ = spool.tile([P, 1], F32, tag="l")
                nc.vector.reduce_sum(l_, lsum, axis=mybir.AxisListType.X)
                linv = spool.tile([P, 1], F32, tag="linv")
                nc.vector.reciprocal(linv, l_)

                po = psO.tile([P, D], F32, tag="o")
                nk = ext // P
                for k in range(nk):
                    pt_ps = psT.tile([P, P], BF16, tag="pt")
                    nc.tensor.transpose(pt_ps, p_sb[:, k * P:(k + 1) * P], ident)
                    pt = ptpool.tile([P, P], BF16, tag="ptsb")
                    nc.vector.tensor_copy(pt, pt_ps)
                    for c in range(D // 512):
                        nc.tensor.matmul(
                            po[:, c * 512:(c + 1) * 512], pt,
                            v[:, k, c * 512:(c + 1) * 512],
                            start=(k == 0), stop=(k == nk - 1))
                o_sb = apool.tile([P, D], F32, tag="o")
                for c in range(D // 512):
                    nc.vector.tensor_scalar_mul(
                        o_sb[:, c * 512:(c + 1) * 512],
                        po[:, c * 512:(c + 1) * 512], linv)
                nc.sync.dma_start(out[j * P:(j + 1) * P, :], o_sb)
        outer.__exit__(None, None, None)


_PROG = None


def _get_prog():
    global _PROG
    if _PROG is None:
        nc = bacc.Bacc("TRN2", target_bir_lowering=False, debug=False,
                       enable_asserts=False)
        x = nc.dram_tensor("x", (S, D), F32, kind="ExternalInput").ap()
        xq = nc.dram_tensor("xq", (QL, D), F32, kind="ExternalInput").ap()
        wq = nc.dram_tensor("wq", (D, D), F32, kind="ExternalInput").ap()
        wk = nc.dram_tensor("wk", (D, D), F32, kind="ExternalInput").ap()
        wv = nc.dram_tensor("wv", (D, D), F32, kind="ExternalInput").ap()
        mask = nc.dram_tensor("mask", (P, 256), F32, kind="ExternalInput").ap()
        out = nc.dram_tensor("out", (QL, D), F32, kind="ExternalOutput").ap()
        with tile.TileContext(nc) as tc:
            _body(tc, x, xq, wq, wk, wv, mask, out)
        nc.compile()
        _PROG = nc
    return _PROG


def _mask_np(h):
    r = np.arange(P)[:, None]
    c = np.arange(P)[None, :]
    tri = np.where(c <= r, 0.0, NEG).astype(np.float32)
    m = np.zeros((P, 256), np.float32)
    if h == 0:
        m[:, :P] = tri
        m[:, P:] = NEG
    else:
        m[:, P:] = tri
    return m


def _in_map_for_core(inputs, core):
    b, h = core // 2, core % 2
    xb = np.ascontiguousarray(np.asarray(inputs["x"], np.float32)[b])
    xqb = np.ascontiguousarray(xb.reshape(NQT, 2, P, D)[:, h].reshape(QL, D))
    return {
        "x": xb,
        "xq": xqb,
        "wq": np.ascontiguousarray(np.asarray(inputs["wq"], np.float32)),
        "wk": np.ascontiguousarray(np.asarray(inputs["wk"], np.float32)),
        "wv": np.ascontiguousarray(np.asarray(inputs["wv"], np.float32)),
        "mask": _mask_np(h),
    }


def _run(inputs, trace=False, tmpdir=None):
    nc = _get_prog()
    in_maps = [_in_map_for_core(inputs, c) for c in range(NCORES)]
    try:
        res = run_bass_kernel_spmd(nc, in_maps, core_ids=list(range(NCORES)),
                                   trace=trace, tmpdir=tmpdir)
    except Exception:
        # first execution of a fresh NEFF occasionally trips a transient
        # device error on this stack; one retry has always succeeded
        res = run_bass_kernel_spmd(nc, in_maps, core_ids=list(range(NCORES)),
                                   trace=trace, tmpdir=tmpdir)
    outf = np.empty((B, S, D), np.float32)
    for core in range(NCORES):
        b, h = core // 2, core % 2
        o = np.asarray(res.results[core]["out"], np.float32)
        outf[b].reshape(NQT, 2, P, D)[:, h] = o.reshape(NQT, P, D)
    return outf, res


def kernel(x, wq, wk, wv):
    outf, _ = _run({"x": x, "wq": wq, "wk": wk, "wv": wv}, trace=False)
    return outf

